# revision 1
# baseline (speedup 1.0000x reference)
"""Trainium2 Bass kernel for CoreferenceResolution.

Math: logits[b,p] = relu(concat(M[b,i], M[b,j], ED[e]) @ W1 + b1) @ W2 + b2
Decomposed as: relu(U[b,i] + V[b,j] + E'[e]) @ W2 + b2 with
  U = M @ W1[:768], V = M @ W1[768:1536], E' = ED @ W1[1536:] + b1
  (b1 folded into E' by appending an all-ones row to ED^T and b1 to W1c).

All indexed lookups run on the TensorEngine as one-hot matmuls in a
transposed layout (preH^T[h, pair] accumulated in PSUM): the three adds fuse
into PSUM accumulation and relu fuses into the PSUM drain on ScalarE.
One-hot masks are built on-device per tile: PE broadcasts a per-column
lane-id row (K=1 matmul with a ones vector) into PSUM, then VectorE
is_equal against an iota per-partition scalar produces the bf16 mask.

Static structure (8 cores = 2 batches x 4 V-buckets):
 - pairs go to the core owning b's mention chunk-of-512; each core's mention
   table is host-reordered so its V bucket is rows 0..511 (V = 4 static
   chunk slots, and V is only projected for those 512 mentions).
 - within a core, pairs are placed into per-a-chunk quota ranges so each
   512-pair tile needs only the 1-2 statically-known U chunks covering its
   quota window; overflow goes to one slop tile with all 16 U slots.
 - E' spans 3 static chunks.
Host-side work is index marshalling only: per-slot lane values (bf16 codes
0..127, 255 = no match), mention reorder, and bf16 casts of the weights
(the kernel computes in bf16 regardless).
"""

import math
import sys

sys.path.insert(0, "/opt/trn_rl_repo")

import numpy as np

HIDDEN = 768
HC = 6                        # hidden chunks of 128
B = 2
N_MENT = 2000
MENT_PAD = 2048
M_CHUNKS = 16
N_PAIRS = 40000
ED_COUNT = 300
ED_PAD = 384
E_CHUNKS = 3
META = 25
W1_ROWS_PAD = 1664            # 1561 -> 13 chunks of 128
W1_CHUNKS = 13
N_CORES = 8
SLICES = 4                    # V buckets (of 512 mentions) per batch
V_CHUNKS = 4                  # mention chunks per V bucket
T = 512                       # pairs per tile

N_EXP = 10240                 # expected pairs per core
NOMATCH = 255.0               # lane code that matches no partition


def _quotas():
    """Per-a-chunk quota (same for every core; mean + 2.5 sigma slack)."""
    qs = []
    for c in range(M_CHUNKS):
        size = min(128, max(0, N_MENT - c * 128))
        p = size / N_MENT
        mean = N_EXP * p
        qs.append(int(math.ceil(mean + 2.5 * math.sqrt(mean))))
    return qs


QUOTAS = _quotas()
QCUM = [0]
for q in QUOTAS:
    QCUM.append(QCUM[-1] + q)
NT_Q = (QCUM[-1] + T - 1) // T        # quota tiles
NT_ALL = NT_Q + 1                     # + one slop tile (all 16 U chunks)
SLOP_CAP = T


def _tile_windows():
    wins = []
    for t in range(NT_Q):
        lo, hi = t * T, (t + 1) * T
        w = [c for c in range(M_CHUNKS) if QCUM[c] < hi and QCUM[c + 1] > lo]
        wins.append(w)
    wins.append(list(range(M_CHUNKS)))  # slop tile
    return wins


WINDOWS = _tile_windows()

# flat static slot list: (tile, kind, chunk); kind: 0=U, 1=V, 2=E
SLOTS = []
SLOT_BASE = []
for t in range(NT_ALL):
    SLOT_BASE.append(len(SLOTS))
    for c in WINDOWS[t]:
        SLOTS.append((t, 0, c))
    for j in range(V_CHUNKS):
        SLOTS.append((t, 1, j))
    for j in range(E_CHUNKS):
        SLOTS.append((t, 2, j))
SLOT_BASE.append(len(SLOTS))
N_SLOTS = len(SLOTS)

_COMPILED = None


def _build(phases="pd", reps=1):
    import concourse.mybir as mybir
    import concourse.tile as tile
    from concourse import bacc
    from concourse.bass import ts

    dt = mybir.dt
    nc = bacc.Bacc("TRN2", target_bir_lowering=False, debug=False,
                   num_devices=N_CORES)

    ments_d = nc.dram_tensor("ments", [MENT_PAD, HIDDEN], dt.bfloat16,
                             kind="ExternalInput").ap()
    w1_d = nc.dram_tensor("w1p", [W1_ROWS_PAD, HIDDEN], dt.bfloat16,
                          kind="ExternalInput").ap()
    w2b_d = nc.dram_tensor("w2b", [128, HC], dt.bfloat16,
                           kind="ExternalInput").ap()
    b2_d = nc.dram_tensor("b2", [1], dt.float32, kind="ExternalInput").ap()
    edt_d = nc.dram_tensor("edt", [32, ED_PAD], dt.bfloat16,
                           kind="ExternalInput").ap()
    vals_d = nc.dram_tensor("vals", [1, N_SLOTS * T], dt.bfloat16,
                            kind="ExternalInput").ap()
    iota_d = nc.dram_tensor("iota", [128, 1], dt.float32,
                            kind="ExternalInput").ap()
    out_d = nc.dram_tensor("out", [NT_ALL * T], dt.float32,
                           kind="ExternalOutput").ap()

    MAXNS = max(SLOT_BASE[t + 1] - SLOT_BASE[t] for t in range(NT_ALL))

    with tile.TileContext(nc) as tc:
        with (
            tc.tile_pool(name="const", bufs=1) as cpool,
            tc.tile_pool(name="tables", bufs=1) as tpool,
        ):
            w1_sb = cpool.tile([128, W1_CHUNKS, HIDDEN], dt.bfloat16)
            w2b = cpool.tile([128, HC], dt.bfloat16)
            b2_sb = cpool.tile([1, 1], dt.float32)
            edt_sb = cpool.tile([32, ED_PAD], dt.bfloat16)
            iota_sb = cpool.tile([128, 1], dt.float32)
            ones_sb = cpool.tile([1, 128], dt.bfloat16)

            u_sb = tpool.tile([128, M_CHUNKS * HIDDEN], dt.bfloat16)
            v_sb = tpool.tile([128, V_CHUNKS * HIDDEN], dt.bfloat16)
            e_sb = tpool.tile([128, E_CHUNKS * HIDDEN], dt.bfloat16)

            nc.sync.dma_start(b2_sb[:], b2_d[:])
            nc.sync.dma_start(w2b[:], w2b_d[:])
            nc.sync.dma_start(edt_sb[:], edt_d[:])
            nc.sync.dma_start(iota_sb[:], iota_d[:])
            nc.vector.memset(ones_sb[:], 1.0)
            nc.sync.dma_start(
                w1_sb[:], w1_d.rearrange("(c p) h -> p c h", p=128))

            for _rep in range(reps):
              with (
                tc.tile_pool(name="mentT", bufs=1) as mtpool,
                tc.tile_pool(name="psA", bufs=4, space="PSUM") as psA,
              ):
                mentT = []
                for k in range(HC):
                    mt = mtpool.tile([128, MENT_PAD], dt.bfloat16,
                                     tag=f"mt{k}", name=f"mentT{k}")
                    nc.sync.dma_start(mt[:], ments_d[:, ts(k, 128)],
                                      transpose=True)
                    mentT.append(mt)

                # ---- E' = [ed^T; 1].T @ [W1c; b1]  (26 contraction rows) ----
                for m in range(E_CHUNKS if "p" in phases else 0):
                    p5 = psA.tile([128, 512], dt.float32, tag="p5")
                    p2 = psA.tile([128, 256], dt.float32, tag="p2")
                    lhs = edt_sb[:META + 1, ts(m, 128)]
                    nc.tensor.matmul(p5[:], lhs, w1_sb[:META + 1, 12, :512],
                                     start=True, stop=True)
                    nc.tensor.matmul(p2[:], lhs, w1_sb[:META + 1, 12, 512:],
                                     start=True, stop=True)
                    nc.vector.tensor_copy(e_sb[:, m * HIDDEN:m * HIDDEN + 512],
                                          p5[:])
                    nc.vector.tensor_copy(
                        e_sb[:, m * HIDDEN + 512:(m + 1) * HIDDEN], p2[:])

                # ---- U (16 chunks) and V (first 4 chunks) projections ----
                for r in range(M_CHUNKS if "p" in phases else 0):
                    u5 = psA.tile([128, 512], dt.float32, tag="p5")
                    u2 = psA.tile([128, 256], dt.float32, tag="p2")
                    do_v = r < V_CHUNKS
                    if do_v:
                        v5 = psA.tile([128, 512], dt.float32, tag="p5")
                        v2 = psA.tile([128, 256], dt.float32, tag="p2")
                    for k in range(HC):
                        lhs = mentT[k][:, ts(r, 128)]
                        st0, sp1 = (k == 0), (k == HC - 1)
                        nc.tensor.matmul(u5[:], lhs, w1_sb[:, k, :512],
                                         start=st0, stop=sp1)
                        nc.tensor.matmul(u2[:], lhs, w1_sb[:, k, 512:],
                                         start=st0, stop=sp1)
                        if do_v:
                            nc.tensor.matmul(v5[:], lhs, w1_sb[:, 6 + k, :512],
                                             start=st0, stop=sp1)
                            nc.tensor.matmul(v2[:], lhs, w1_sb[:, 6 + k, 512:],
                                             start=st0, stop=sp1)
                    ro = r * HIDDEN
                    nc.vector.tensor_copy(u_sb[:, ro:ro + 512], u5[:])
                    nc.vector.tensor_copy(u_sb[:, ro + 512:ro + HIDDEN], u2[:])
                    if do_v:
                        nc.scalar.copy(v_sb[:, ro:ro + 512], v5[:])
                        nc.scalar.copy(v_sb[:, ro + 512:ro + HIDDEN], v2[:])

            # ---- pair tiles: build one-hots + expand + relu + dot ----
              with (
                  tc.tile_pool(name="oh", bufs=2) as ohpool,
                  tc.tile_pool(name="vt", bufs=2) as vtpool,
                  tc.tile_pool(name="h", bufs=6) as hpool,
                  tc.tile_pool(name="o", bufs=2) as opool,
                  tc.tile_pool(name="psD", bufs=4, space="PSUM") as psD,
                  tc.tile_pool(name="psB", bufs=2, space="PSUM") as psB,
                  tc.tile_pool(name="psL", bufs=2, space="PSUM") as psL,
              ):
                  relu = mybir.ActivationFunctionType.Relu
                  ident = mybir.ActivationFunctionType.Identity
                  eq = mybir.AluOpType.is_equal
                  if "d" not in phases:
                      for t in range(NT_ALL):
                          lt = opool.tile([1, T], dt.float32, tag="lt")
                          nc.vector.memset(lt[:], 0.0)
                          nc.sync.dma_start(out_d[ts(t, T)], lt[:])
                  for t in range(NT_ALL if "d" in phases else 0):
                      base = SLOT_BASE[t]
                      ns = SLOT_BASE[t + 1] - base
                      vt = vtpool.tile([1, MAXNS, T], dt.bfloat16, tag="vt")
                      nc.sync.dma_start(
                          vt[:1, :ns, :],
                          vals_d[:, base * T:(base + ns) * T]
                          .rearrange("o (s c) -> o s c", c=T))
                      oh_t = ohpool.tile([128, MAXNS, T], dt.bfloat16, tag="oh")
                      for s in range(ns):
                          pb = psB.tile([128, T], dt.float32, tag="pb")
                          nc.tensor.matmul(pb[:], ones_sb[:], vt[:1, s, :],
                                           start=True, stop=True)
                          nc.vector.tensor_scalar(oh_t[:, s, :], pb[:],
                                                  iota_sb[:], None, eq)
                      pl = psL.tile([1, T], dt.float32, tag="pl")
                      for hc in range(HC):
                          ph = psD.tile([128, T], dt.float32, tag="ph")
                          for s in range(ns):
                              _, kind, c = SLOTS[base + s]
                              tab = (u_sb, v_sb, e_sb)[kind]
                              lhs = tab[:, c * HIDDEN + hc * 128:
                                        c * HIDDEN + (hc + 1) * 128]
                              nc.tensor.matmul(ph[:], lhs, oh_t[:, s, :],
                                               start=(s == 0), stop=(s == ns - 1))
                          h_sb = hpool.tile([128, T], dt.bfloat16, tag="h")
                          nc.scalar.activation(h_sb[:], ph[:], relu)
                          nc.tensor.matmul(pl[:], w2b[:, hc:hc + 1], h_sb[:],
                                           start=(hc == 0), stop=(hc == HC - 1))
                      lt = opool.tile([1, T], dt.float32, tag="lt")
                      nc.scalar.activation(lt[:], pl[:], ident,
                                           bias=b2_sb[:1, :1])
                      nc.sync.dma_start(out_d[ts(t, T)], lt[:])

    nc.compile()
    return nc


def _get_compiled():
    global _COMPILED
    if _COMPILED is None:
        _COMPILED = _build()
    return _COMPILED


def _assign(core_pairs_a):
    """Place pairs into quota slots by a-chunk; overflow -> slop tile."""
    n = len(core_pairs_a)
    pos = np.full(n, -1, np.int64)
    ah = core_pairs_a // 128
    slop_next = NT_Q * T
    for c in range(M_CHUNKS):
        idx = np.nonzero(ah == c)[0]
        k = min(len(idx), QUOTAS[c])
        pos[idx[:k]] = QCUM[c] + np.arange(k)
        for i in idx[k:]:
            assert slop_next < NT_Q * T + SLOP_CAP, "slop overflow"
            pos[i] = slop_next
            slop_next += 1
    return pos


_SLOT_OF = {(t, kind, c): s for s, (t, kind, c) in enumerate(SLOTS)}


def make_in_maps(mention_reprs, coref_mention_pairs, coref_eds, ed_table,
                 W1, b1, W2, b2):
    import ml_dtypes

    bf16 = ml_dtypes.bfloat16
    mention_reprs = np.asarray(mention_reprs, dtype=np.float32)
    pairs = np.asarray(coref_mention_pairs).astype(np.int64)
    eds = np.asarray(coref_eds).astype(np.int64)
    W1 = np.asarray(W1, dtype=np.float32)
    W2 = np.asarray(W2, dtype=np.float32)
    b1 = np.asarray(b1, dtype=np.float32).reshape(HIDDEN)
    b2 = np.asarray(b2, dtype=np.float32)
    ed_table = np.asarray(ed_table, dtype=np.float32)

    w1p = np.zeros((W1_ROWS_PAD, HIDDEN), np.float32)
    w1p[:W1.shape[0]] = W1
    w1p[W1.shape[0]] = b1                      # b1 folded (row 1561)
    edt = np.zeros((32, ED_PAD), np.float32)
    edt[:META, :ed_table.shape[0]] = ed_table.T
    edt[META, :] = 1.0                         # ones row -> picks up b1
    w2b = np.ascontiguousarray(W2.reshape(HC, 128).T)  # [p, c] = W2[c*128+p]
    iota = np.arange(128, dtype=np.float32).reshape(128, 1)

    shared = {
        "w1p": w1p.astype(bf16),
        "w2b": w2b.astype(bf16),
        "b2": b2.reshape(1),
        "edt": edt.astype(bf16),
        "iota": iota,
    }

    in_maps = []
    placements = []
    for core in range(N_CORES):
        b = core // SLICES
        q = core % SLICES
        bucket = np.arange(512 * q, min(512 * (q + 1), N_MENT))
        rest = np.concatenate([np.arange(0, 512 * q),
                               np.arange(min(512 * (q + 1), N_MENT), N_MENT)])
        perm = np.concatenate([bucket, rest])
        inv_perm = np.empty(N_MENT, np.int64)
        inv_perm[perm] = np.arange(N_MENT)

        ments = np.zeros((MENT_PAD, HIDDEN), np.float32)
        ments[:N_MENT] = mention_reprs[b][perm]

        bsel = (pairs[b, :, 1] >= 512 * q) & (pairs[b, :, 1] < 512 * (q + 1))
        psel = np.nonzero(bsel)[0]
        a_new = inv_perm[pairs[b, psel, 0]]
        b_loc = inv_perm[pairs[b, psel, 1]]
        e_val = eds[b, psel]

        pos = _assign(a_new)
        tile_i = pos // T
        col_i = pos % T

        vals = np.full((N_SLOTS, T), NOMATCH, np.float32)
        su = np.array([_SLOT_OF[(t, 0, c)]
                       for t, c in zip(tile_i, a_new // 128)])
        sv = np.array([_SLOT_OF[(t, 1, c)]
                       for t, c in zip(tile_i, b_loc // 128)])
        se = np.array([_SLOT_OF[(t, 2, c)]
                       for t, c in zip(tile_i, e_val // 128)])
        vals[su, col_i] = a_new % 128
        vals[sv, col_i] = b_loc % 128
        vals[se, col_i] = e_val % 128

        placements.append((psel, b, pos))
        in_maps.append({"ments": ments.astype(bf16),
                        "vals": vals.reshape(1, -1).astype(bf16),
                        **shared})
    make_in_maps.placements = placements
    return in_maps


def unshard(results, placements):
    out = np.zeros((B, N_PAIRS), np.float32)
    for core in range(N_CORES):
        psel, b, pos = placements[core]
        vals = results[core]["out"]
        out[b, psel] = vals[pos]
    return out


def kernel(**inputs):
    from concourse.bass_utils import run_bass_kernel_spmd

    nc = _get_compiled()
    in_maps = make_in_maps(**inputs)
    placements = make_in_maps.placements
    res = run_bass_kernel_spmd(nc, in_maps, list(range(N_CORES)))
    return unshard(res.results, placements)



# revision 4
# speedup vs baseline: 5.3471x; 5.3471x over previous
"""Trainium2 Bass kernel for CoreferenceResolution.

Math: logits[b,p] = relu(concat(M[b,i], M[b,j], ED[e]) @ W1 + b1) @ W2 + b2
Decomposed as: relu(U[b,i] + V[b,j] + E'[e]) @ W2 + b2 with
  U = M @ W1[:768], V = M @ W1[768:1536], E' = ED @ W1[1536:] + b1
  (b1 folded into E' by appending an all-ones row to ED^T and b1 to W1c).

All indexed lookups run on the TensorEngine as one-hot matmuls in a
transposed layout (preH^T[h, pair] accumulated in PSUM): the three adds fuse
into PSUM accumulation and relu fuses into the PSUM drain on ScalarE.
One-hot masks are built on-device per tile: PE broadcasts a per-column
lane-id row (K=1 matmul with a ones vector) into PSUM, then VectorE
is_equal against an iota per-partition scalar produces the bf16 mask.

Static structure (8 cores = 2 batches x 4 V-buckets):
 - pairs go to the core owning b's mention chunk-of-512; each core's mention
   table is host-reordered so its V bucket is rows 0..511 (V = 4 static
   chunk slots, and V is only projected for those 512 mentions).
 - within a core, pairs are placed into per-a-chunk quota ranges so each
   512-pair tile needs only the 1-2 statically-known U chunks covering its
   quota window; overflow goes to one slop tile with all 16 U slots.
 - E' spans 3 static chunks.

Transport: the axon tunnel charges ~0.8 ms per input tensor per execution,
so ALL per-core inputs are packed into ONE flat bf16 blob (iota is generated
on device; f32 b2 ships as a hi/lo bf16 pair recombined on device).
"""

import math
import sys

sys.path.insert(0, "/opt/trn_rl_repo")

import numpy as np

HIDDEN = 768
HC = 6                        # hidden chunks of 128
B = 2
N_MENT = 2000
MENT_PAD = 2048
M_CHUNKS = 16
N_PAIRS = 40000
ED_COUNT = 300
ED_PAD = 384
E_CHUNKS = 3
META = 25
W1_ROWS_PAD = 1664            # 1561 -> 13 chunks of 128
W1_CHUNKS = 13
N_CORES = 8
SLICES = 4                    # V buckets (of 512 mentions) per batch
V_CHUNKS = 4                  # mention chunks per V bucket
T = 512                       # pairs per tile

N_EXP = 10240                 # expected pairs per core
NOMATCH = 255.0               # lane code that matches no partition


def _quotas():
    """Per-a-chunk quota (same for every core; mean + 2.5 sigma slack)."""
    qs = []
    for c in range(M_CHUNKS):
        size = min(128, max(0, N_MENT - c * 128))
        p = size / N_MENT
        mean = N_EXP * p
        qs.append(int(math.ceil(mean + 2.5 * math.sqrt(mean))))
    return qs


QUOTAS = _quotas()
QCUM = [0]
for q in QUOTAS:
    QCUM.append(QCUM[-1] + q)
NT_Q = (QCUM[-1] + T - 1) // T        # quota tiles
NT_ALL = NT_Q + 1                     # + one slop tile (all 16 U chunks)
SLOP_CAP = T


def _tile_windows():
    wins = []
    for t in range(NT_Q):
        lo, hi = t * T, (t + 1) * T
        w = [c for c in range(M_CHUNKS) if QCUM[c] < hi and QCUM[c + 1] > lo]
        wins.append(w)
    wins.append(list(range(M_CHUNKS)))  # slop tile
    return wins


WINDOWS = _tile_windows()

# flat static slot list: (tile, kind, chunk); kind: 0=U, 1=V, 2=E
SLOTS = []
SLOT_BASE = []
for t in range(NT_ALL):
    SLOT_BASE.append(len(SLOTS))
    for c in WINDOWS[t]:
        SLOTS.append((t, 0, c))
    for j in range(V_CHUNKS):
        SLOTS.append((t, 1, j))
    for j in range(E_CHUNKS):
        SLOTS.append((t, 2, j))
SLOT_BASE.append(len(SLOTS))
N_SLOTS = len(SLOTS)

# ---- single-blob input layout (element offsets into flat bf16 blob) ----
MENTS_SZ = MENT_PAD * HIDDEN
W1_SZ = W1_ROWS_PAD * HIDDEN
EDT_SZ = 32 * ED_PAD
W2B_SZ = 128 * HC
MENTS_OFF = 0
W1_OFF = MENTS_OFF + MENTS_SZ
EDT_OFF = W1_OFF + W1_SZ
W2B_OFF = EDT_OFF + EDT_SZ
B2_OFF = W2B_OFF + W2B_SZ
VALS_OFF = B2_OFF + 512
BLOB_SZ = VALS_OFF + N_SLOTS * T

_COMPILED = None


def _build(phases="pd", reps=1):
    import concourse.mybir as mybir
    import concourse.tile as tile
    from concourse import bacc
    from concourse.bass import ts

    dt = mybir.dt
    nc = bacc.Bacc("TRN2", target_bir_lowering=False, debug=False,
                   num_devices=N_CORES)

    blob_d = nc.dram_tensor("blob", [BLOB_SZ], dt.bfloat16,
                            kind="ExternalInput").ap()
    out_d = nc.dram_tensor("out", [NT_ALL * T], dt.float32,
                           kind="ExternalOutput").ap()

    ments_2d = blob_d[MENTS_OFF:MENTS_OFF + MENTS_SZ].rearrange(
        "(r h) -> r h", h=HIDDEN)

    MAXNS = max(SLOT_BASE[t + 1] - SLOT_BASE[t] for t in range(NT_ALL))

    with tile.TileContext(nc) as tc:
        with (
            tc.tile_pool(name="const", bufs=1) as cpool,
            tc.tile_pool(name="tables", bufs=1) as tpool,
        ):
            w1_sb = cpool.tile([128, W1_CHUNKS, HIDDEN], dt.bfloat16)
            w2b = cpool.tile([128, HC], dt.bfloat16)
            b2hl = cpool.tile([1, 2], dt.bfloat16)
            b2_sb = cpool.tile([1, 1], dt.float32)
            edt_sb = cpool.tile([32, ED_PAD], dt.bfloat16)
            iota_sb = cpool.tile([128, 1], dt.float32)
            ones_sb = cpool.tile([1, 128], dt.bfloat16)

            u_sb = tpool.tile([128, M_CHUNKS * HIDDEN], dt.bfloat16)
            v_sb = tpool.tile([128, V_CHUNKS * HIDDEN], dt.bfloat16)
            e_sb = tpool.tile([128, E_CHUNKS * HIDDEN], dt.bfloat16)

            nc.sync.dma_start(
                b2hl[:], blob_d[B2_OFF:B2_OFF + 2].rearrange("(o c) -> o c", o=1))
            b2f32 = cpool.tile([1, 2], dt.float32)
            nc.vector.tensor_copy(b2f32[:], b2hl[:])
            nc.vector.tensor_scalar(b2_sb[:], b2f32[:1, :1], b2f32[:1, 1:2],
                                    None, mybir.AluOpType.add)
            nc.sync.dma_start(
                w2b[:], blob_d[W2B_OFF:W2B_OFF + W2B_SZ].rearrange(
                    "(p c) -> p c", p=128))
            nc.sync.dma_start(
                edt_sb[:], blob_d[EDT_OFF:EDT_OFF + EDT_SZ].rearrange(
                    "(p c) -> p c", p=32))
            nc.gpsimd.iota(iota_sb[:], [[1, 1]], base=0, channel_multiplier=1,
                           allow_small_or_imprecise_dtypes=True)
            nc.vector.memset(ones_sb[:], 1.0)
            nc.sync.dma_start(
                w1_sb[:], blob_d[W1_OFF:W1_OFF + W1_SZ].rearrange(
                    "(c p h) -> p c h", p=128, h=HIDDEN))

            for _rep in range(reps):
              with (
                tc.tile_pool(name="mentT", bufs=1) as mtpool,
                tc.tile_pool(name="psA", bufs=4, space="PSUM") as psA,
              ):
                mentT = []
                for k in range(HC):
                    mt = mtpool.tile([128, MENT_PAD], dt.bfloat16,
                                     tag=f"mt{k}", name=f"mentT{k}")
                    if "m" in phases or "p" in phases:
                        nc.sync.dma_start(mt[:], ments_2d[:, ts(k, 128)],
                                          transpose=True)
                    mentT.append(mt)

                # ---- E' = [ed^T; 1].T @ [W1c; b1]  (26 contraction rows) ----
                for m in range(E_CHUNKS if "p" in phases else 0):
                    p5 = psA.tile([128, 512], dt.float32, tag="p5")
                    p2 = psA.tile([128, 256], dt.float32, tag="p2")
                    lhs = edt_sb[:META + 1, ts(m, 128)]
                    nc.tensor.matmul(p5[:], lhs, w1_sb[:META + 1, 12, :512],
                                     start=True, stop=True)
                    nc.tensor.matmul(p2[:], lhs, w1_sb[:META + 1, 12, 512:],
                                     start=True, stop=True)
                    nc.vector.tensor_copy(e_sb[:, m * HIDDEN:m * HIDDEN + 512],
                                          p5[:])
                    nc.vector.tensor_copy(
                        e_sb[:, m * HIDDEN + 512:(m + 1) * HIDDEN], p2[:])

                # ---- U (16 chunks) and V (first 4 chunks) projections ----
                for r in range(M_CHUNKS if "p" in phases else 0):
                    u5 = psA.tile([128, 512], dt.float32, tag="p5")
                    u2 = psA.tile([128, 256], dt.float32, tag="p2")
                    do_v = r < V_CHUNKS
                    if do_v:
                        v5 = psA.tile([128, 512], dt.float32, tag="p5")
                        v2 = psA.tile([128, 256], dt.float32, tag="p2")
                    for k in range(HC):
                        lhs = mentT[k][:, ts(r, 128)]
                        st0, sp1 = (k == 0), (k == HC - 1)
                        nc.tensor.matmul(u5[:], lhs, w1_sb[:, k, :512],
                                         start=st0, stop=sp1)
                        nc.tensor.matmul(u2[:], lhs, w1_sb[:, k, 512:],
                                         start=st0, stop=sp1)
                        if do_v:
                            nc.tensor.matmul(v5[:], lhs, w1_sb[:, 6 + k, :512],
                                             start=st0, stop=sp1)
                            nc.tensor.matmul(v2[:], lhs, w1_sb[:, 6 + k, 512:],
                                             start=st0, stop=sp1)
                    ro = r * HIDDEN
                    nc.vector.tensor_copy(u_sb[:, ro:ro + 512], u5[:])
                    nc.vector.tensor_copy(u_sb[:, ro + 512:ro + HIDDEN], u2[:])
                    if do_v:
                        nc.scalar.copy(v_sb[:, ro:ro + 512], v5[:])
                        nc.scalar.copy(v_sb[:, ro + 512:ro + HIDDEN], v2[:])

            # ---- pair tiles: build one-hots + expand + relu + dot ----
              with (
                  tc.tile_pool(name="oh", bufs=2) as ohpool,
                  tc.tile_pool(name="vt", bufs=2) as vtpool,
                  tc.tile_pool(name="h", bufs=6) as hpool,
                  tc.tile_pool(name="o", bufs=2) as opool,
                  tc.tile_pool(name="psD", bufs=4, space="PSUM") as psD,
                  tc.tile_pool(name="psB", bufs=2, space="PSUM") as psB,
                  tc.tile_pool(name="psL", bufs=2, space="PSUM") as psL,
              ):
                  relu = mybir.ActivationFunctionType.Relu
                  ident = mybir.ActivationFunctionType.Identity
                  eq = mybir.AluOpType.is_equal
                  if "d" not in phases:
                      for t in range(NT_ALL):
                          lt = opool.tile([1, T], dt.float32, tag="lt")
                          nc.vector.memset(lt[:], 0.0)
                          nc.sync.dma_start(out_d[ts(t, T)], lt[:])
                  for t in range(NT_ALL if "d" in phases else 0):
                      base = SLOT_BASE[t]
                      ns = SLOT_BASE[t + 1] - base
                      vt = vtpool.tile([1, MAXNS, T], dt.bfloat16, tag="vt")
                      nc.sync.dma_start(
                          vt[:1, :ns, :],
                          blob_d[VALS_OFF + base * T:VALS_OFF + (base + ns) * T]
                          .rearrange("(o s c) -> o s c", o=1, c=T))
                      oh_t = ohpool.tile([128, MAXNS, T], dt.bfloat16, tag="oh")
                      for s in range(ns):
                          pb = psB.tile([128, T], dt.float32, tag="pb")
                          nc.tensor.matmul(pb[:], ones_sb[:], vt[:1, s, :],
                                           start=True, stop=True)
                          nc.vector.tensor_scalar(oh_t[:, s, :], pb[:],
                                                  iota_sb[:], None, eq)
                      pl = psL.tile([1, T], dt.float32, tag="pl")
                      for hc in range(HC):
                          ph = psD.tile([128, T], dt.float32, tag="ph")
                          for s in range(ns):
                              _, kind, c = SLOTS[base + s]
                              tab = (u_sb, v_sb, e_sb)[kind]
                              lhs = tab[:, c * HIDDEN + hc * 128:
                                        c * HIDDEN + (hc + 1) * 128]
                              nc.tensor.matmul(ph[:], lhs, oh_t[:, s, :],
                                               start=(s == 0), stop=(s == ns - 1))
                          h_sb = hpool.tile([128, T], dt.bfloat16, tag="h")
                          nc.scalar.activation(h_sb[:], ph[:], relu)
                          nc.tensor.matmul(pl[:], w2b[:, hc:hc + 1], h_sb[:],
                                           start=(hc == 0), stop=(hc == HC - 1))
                      lt = opool.tile([1, T], dt.float32, tag="lt")
                      nc.scalar.activation(lt[:], pl[:], ident,
                                           bias=b2_sb[:1, :1])
                      nc.sync.dma_start(out_d[ts(t, T)], lt[:])

    nc.compile()
    return nc


def _get_compiled():
    global _COMPILED
    if _COMPILED is None:
        _COMPILED = _build()
    return _COMPILED


def _assign(core_pairs_a):
    """Place pairs into quota slots by a-chunk; overflow -> slop tile."""
    n = len(core_pairs_a)
    pos = np.full(n, -1, np.int64)
    ah = core_pairs_a // 128
    slop_next = NT_Q * T
    for c in range(M_CHUNKS):
        idx = np.nonzero(ah == c)[0]
        k = min(len(idx), QUOTAS[c])
        pos[idx[:k]] = QCUM[c] + np.arange(k)
        for i in idx[k:]:
            assert slop_next < NT_Q * T + SLOP_CAP, "slop overflow"
            pos[i] = slop_next
            slop_next += 1
    return pos


_SLOT_OF = {(t, kind, c): s for s, (t, kind, c) in enumerate(SLOTS)}


def make_in_maps(mention_reprs, coref_mention_pairs, coref_eds, ed_table,
                 W1, b1, W2, b2):
    import ml_dtypes

    bf16 = ml_dtypes.bfloat16
    mention_reprs = np.asarray(mention_reprs, dtype=np.float32)
    pairs = np.asarray(coref_mention_pairs).astype(np.int64)
    eds = np.asarray(coref_eds).astype(np.int64)
    W1 = np.asarray(W1, dtype=np.float32)
    W2 = np.asarray(W2, dtype=np.float32)
    b1 = np.asarray(b1, dtype=np.float32).reshape(HIDDEN)
    b2 = np.asarray(b2, dtype=np.float32)
    ed_table = np.asarray(ed_table, dtype=np.float32)

    w1p = np.zeros((W1_ROWS_PAD, HIDDEN), np.float32)
    w1p[:W1.shape[0]] = W1
    w1p[W1.shape[0]] = b1                      # b1 folded (row 1561)
    edt = np.zeros((32, ED_PAD), np.float32)
    edt[:META, :ed_table.shape[0]] = ed_table.T
    edt[META, :] = 1.0                         # ones row -> picks up b1
    w2b = np.ascontiguousarray(W2.reshape(HC, 128).T)  # [p, c] = W2[c*128+p]

    b2f = np.float32(b2.reshape(-1)[0] if b2.size else 0.0)
    b2hi = np.float32(bf16(b2f))
    b2lo = np.float32(bf16(np.float32(b2f - b2hi)))

    shared = np.zeros(VALS_OFF, bf16)
    shared[W1_OFF:W1_OFF + W1_SZ] = w1p.astype(bf16).reshape(-1)
    shared[EDT_OFF:EDT_OFF + EDT_SZ] = edt.astype(bf16).reshape(-1)
    shared[W2B_OFF:W2B_OFF + W2B_SZ] = w2b.astype(bf16).reshape(-1)
    shared[B2_OFF] = bf16(b2hi)
    shared[B2_OFF + 1] = bf16(b2lo)

    in_maps = []
    placements = []
    for core in range(N_CORES):
        b = core // SLICES
        q = core % SLICES
        bucket = np.arange(512 * q, min(512 * (q + 1), N_MENT))
        rest = np.concatenate([np.arange(0, 512 * q),
                               np.arange(min(512 * (q + 1), N_MENT), N_MENT)])
        perm = np.concatenate([bucket, rest])
        inv_perm = np.empty(N_MENT, np.int64)
        inv_perm[perm] = np.arange(N_MENT)

        blob = np.zeros(BLOB_SZ, bf16)
        blob[:VALS_OFF] = shared
        blob[MENTS_OFF:MENTS_OFF + N_MENT * HIDDEN] = (
            mention_reprs[b][perm].astype(bf16).reshape(-1))

        bsel = (pairs[b, :, 1] >= 512 * q) & (pairs[b, :, 1] < 512 * (q + 1))
        psel = np.nonzero(bsel)[0]
        a_new = inv_perm[pairs[b, psel, 0]]
        b_loc = inv_perm[pairs[b, psel, 1]]
        e_val = eds[b, psel]

        pos = _assign(a_new)
        tile_i = pos // T
        col_i = pos % T

        vals = np.full((N_SLOTS, T), NOMATCH, np.float32)
        su = np.array([_SLOT_OF[(t, 0, c)]
                       for t, c in zip(tile_i, a_new // 128)])
        sv = np.array([_SLOT_OF[(t, 1, c)]
                       for t, c in zip(tile_i, b_loc // 128)])
        se = np.array([_SLOT_OF[(t, 2, c)]
                       for t, c in zip(tile_i, e_val // 128)])
        vals[su, col_i] = a_new % 128
        vals[sv, col_i] = b_loc % 128
        vals[se, col_i] = e_val % 128
        blob[VALS_OFF:] = vals.reshape(-1).astype(bf16)

        placements.append((psel, b, pos))
        in_maps.append({"blob": blob})
    make_in_maps.placements = placements
    return in_maps


def unshard(results, placements):
    out = np.zeros((B, N_PAIRS), np.float32)
    for core in range(N_CORES):
        psel, b, pos = placements[core]
        vals = results[core]["out"]
        out[b, psel] = vals[pos]
    return out


def kernel(**inputs):
    from concourse.bass_utils import run_bass_kernel_spmd

    nc = _get_compiled()
    in_maps = make_in_maps(**inputs)
    placements = make_in_maps.placements
    res = run_bass_kernel_spmd(nc, in_maps, list(range(N_CORES)))
    return unshard(res.results, placements)
